# revision 10
# baseline (speedup 1.0000x reference)
"""Trainium2 Bass kernel for nn_DeformConv2d_for_Style.

Strategy: data-parallel over batch B=8 across 8 NeuronCores (one image per
core). Per core:
  1. Offset conv (reflect-pad) as 9 accumulating fp16 matmuls per nt-block
     reading a single padded-image tile via 2-D APs, then PE transposes to
     sample-major offT.
  2. Offset processing on DVE ([128, 288] fp32): sampling positions, clamped
     floor, bilinear weights folded with validity masks, int16 gather indices
     into a zero-padded 2x2-patch table in HBM.
  3. dma_gather (fp16, elem=512 = 2x2-patch rows: all four bilinear taps x
     128 ch in one 1KB descriptor) across 4 SWDGE queues, 4-deep buffering.
  4. Bilinear reduce as all-DVE chains (tensor_scalar + 3 scalar_tensor_tensor
     with per-partition (= per-sample) scalar weights).
  5. PE transpose of weighted tiles -> cols [channel, sample], ACT copies out
     of PSUM, main conv as fp16 matmuls (N=256) + bias via ACT, fp16 output.

All static data layout prep (padding, transposes, fp16 casts, index bases,
identities) happens host-side in numpy.
"""

import os
import sys
import dataclasses

import numpy as np

sys.path.insert(0, "/opt/trn_rl_repo")

B, CIN, COUT, K = 8, 128, 256, 3
KK = K * K
H = W = 64
HW = H * W
NT = HW // 128          # 32 l-tiles of 128 samples
G = NT * KK             # 288 groups of 128 samples
PW = W + 2              # 66
TROWS = 4608            # zero-padded transposed table rows (68*66=4488 used)
CHUNK_T = 2             # l-tiles per chunk
NCH = NT // CHUNK_T     # 16 chunks
GC = CHUNK_T * KK       # 18 groups per chunk
SC = GC * 128           # 2304 samples per chunk

_CACHE = {}


def _build_program(loop_n=None, variant=None):
    import concourse.bass as bass
    import concourse.tile as tile
    from concourse import bacc, mybir

    f16 = mybir.dt.float16
    f32 = mybir.dt.float32
    i16 = mybir.dt.int16
    i32 = mybir.dt.int32
    Alu = mybir.AluOpType
    Act = mybir.ActivationFunctionType

    nc = bacc.Bacc("TRN2", target_bir_lowering=False, debug=False,
                   num_swdge_queues=4)

    if variant in ("gonly4q", "gonly1q", "donly", "dhalf"):
        return _build_gather_probe(nc, loop_n, variant)

    # ---- DRAM parameters (per-core) ----
    xpad_d = nc.dram_tensor("xpad", [CIN, PW * PW], f16, kind="ExternalInput")
    xt_d = nc.dram_tensor("xt", [TROWS * 4 * CIN], f16, kind="ExternalInput")
    wofft_d = nc.dram_tensor("wofft", [CIN, KK * 2 * KK], f16, kind="ExternalInput")
    wdct_d = nc.dram_tensor("wdct", [CIN, KK * COUT], f16, kind="ExternalInput")
    bdc_d = nc.dram_tensor("bdc", [128, 2], f32, kind="ExternalInput")
    pyb_d = nc.dram_tensor("pyb", [128, G], f32, kind="ExternalInput")
    pxb_d = nc.dram_tensor("pxb", [128, G], f32, kind="ExternalInput")
    id16_d = nc.dram_tensor("id16", [128, 128], f16, kind="ExternalInput")
    id32_d = nc.dram_tensor("id32", [32, 32], f32, kind="ExternalInput")
    out_d = nc.dram_tensor("out", [COUT, HW], f16, kind="ExternalOutput")

    # gather source view: 2x2-patch rows of 4*128 ch, one row per index
    gsrc = dataclasses.replace(xt_d.ap(), ap=[[4 * CIN, TROWS - 1], [1, 4 * CIN]])

    from contextlib import ExitStack

    with tile.TileContext(nc) as tc, ExitStack() as ctx:
        cpool = ctx.enter_context(tc.tile_pool(name="consts", bufs=2))
        spool = ctx.enter_context(tc.tile_pool(name="scratch", bufs=1))
        wpool = ctx.enter_context(tc.tile_pool(name="wts", bufs=2))
        gpool = ctx.enter_context(tc.tile_pool(name="gath", bufs=4))
        zpool = ctx.enter_context(tc.tile_pool(name="z", bufs=6))
        colpool = ctx.enter_context(tc.tile_pool(name="cols", bufs=2))
        opool = ctx.enter_context(tc.tile_pool(name="outs", bufs=3))
        ps_off = ctx.enter_context(tc.tile_pool(name="ps_off", bufs=1, space="PSUM"))
        ps_tr = ctx.enter_context(tc.tile_pool(name="ps_tr", bufs=2, space="PSUM"))
        ps_mm = ctx.enter_context(tc.tile_pool(name="ps_mm", bufs=2, space="PSUM"))

        from contextlib import nullcontext
        loop_cm = tc.For_i(0, loop_n, 1) if loop_n else nullcontext()
        with loop_cm:
            # ---- load constants ----
            xpad = cpool.tile([CIN, PW * PW], f16, tag="xpad")
            nc.sync.dma_start(xpad[:], xpad_d.ap())
            wofft = cpool.tile([CIN, KK * 2 * KK], f16, tag="wofft")
            nc.sync.dma_start(wofft[:], wofft_d.ap())
            wdct = cpool.tile([CIN, KK * COUT], f16, tag="wdct")
            nc.sync.dma_start(wdct[:], wdct_d.ap())
            bdc = cpool.tile([128, 2], f32, tag="bdc")
            nc.sync.dma_start(bdc[:], bdc_d.ap())
            pyb = cpool.tile([128, G], f32, tag="pyb")
            nc.sync.dma_start(pyb[:], pyb_d.ap())
            pxb = cpool.tile([128, G], f32, tag="pxb")
            nc.sync.dma_start(pxb[:], pxb_d.ap())
            id16 = cpool.tile([128, 128], f16, tag="id16")
            nc.sync.dma_start(id16[:], id16_d.ap())
            id32 = cpool.tile([32, 32], f32, tag="id32")
            nc.sync.dma_start(id32[:], id32_d.ap())

            xp3 = xpad[:].rearrange("p (r q) -> p r q", q=PW)

            # ---- stage 1: offset conv per nt-block, PE transposes to
            # sample-major offT [128 lp, t*18 + ch] ----
            offT = spool.tile([128, NT * 2 * KK], f32, tag="offT")
            for nt in range(8):
                po = ps_off.tile([18, 8, 64], f32, tag="po", name="po")
                for kk in range(KK):
                    ki, kj = kk // 3, kk % 3
                    r0 = nt * 8 + ki
                    nc.tensor.matmul(
                        po[:], wofft[:, kk * 18:(kk + 1) * 18],
                        xp3[:, r0:r0 + 8, kj:kj + 64],
                        start=(kk == 0), stop=(kk == KK - 1),
                    )
                off_sb = opool.tile([18, 512], f32, tag="off_sb")
                nc.scalar.copy(off_sb[:], po[:].rearrange("p a b -> p (a b)"))
                for q in range(4):
                    t = nt * 4 + q
                    pT = ps_off.tile([128, 2 * KK], f32, tag="pT", name="pT")
                    nc.tensor.transpose(
                        pT[:], off_sb[:, q * 128:(q + 1) * 128], id32[0:18, 0:18])
                    nc.scalar.copy(offT[:, t * 18:(t + 1) * 18], pT[:])

            # ---- stage 2: offset processing on [128, G] fp32 ----
            off4 = offT[:].rearrange("p (t k u) -> p t k u", k=KK, u=2)
            dy = off4[:, :, :, 0]
            dx = off4[:, :, :, 1]

            def stile(tag, dt=f32):
                return spool.tile([128, G], dt, tag=tag, name=tag)

            def v3(ap):  # [128, G] -> [128, NT, KK] view to match dy/dx dims
                return ap.rearrange("p (t k) -> p t k", k=KK)

            py = stile("py")
            nc.vector.tensor_tensor(v3(py[:]), dy, v3(pyb[:]), Alu.add)
            px = stile("px")
            nc.vector.tensor_tensor(v3(px[:]), dx, v3(pxb[:]), Alu.add)
            for p in (py, px):
                nc.vector.tensor_scalar(p[:], p[:], -1.0, None, Alu.max)
                nc.vector.tensor_scalar(p[:], p[:], 64.75, None, Alu.min)

            def floor_of(p, tag):
                ti = spool.tile([128, G], i32, tag="fl_i", name="fl_i")
                nc.vector.tensor_copy(ti[:], p[:])
                tf = stile(tag)
                nc.vector.tensor_copy(tf[:], ti[:])
                gt = spool.tile([128, G], f32, tag="fl_g", name="fl_g")
                nc.vector.tensor_tensor(gt[:], tf[:], p[:], Alu.is_gt)
                nc.vector.tensor_tensor(tf[:], tf[:], gt[:], Alu.subtract)
                return tf

            y0 = floor_of(py, "y0")
            x0 = floor_of(px, "x0")
            fy = stile("fy")
            nc.vector.tensor_tensor(fy[:], py[:], y0[:], Alu.subtract)
            fx = stile("fx")
            nc.vector.tensor_tensor(fx[:], px[:], x0[:], Alu.subtract)

            def validity(v, lo, hi):
                a = spool.tile([128, G], f32, tag="val_a", name="val_a")
                nc.vector.tensor_scalar(a[:], v[:], float(lo), None, Alu.is_ge)
                b = spool.tile([128, G], f32, tag="val_b", name="val_b")
                nc.vector.tensor_scalar(b[:], v[:], float(hi), None, Alu.is_le)
                nc.vector.tensor_tensor(a[:], a[:], b[:], Alu.mult)
                return a

            def wtile(tag):
                return wpool.tile([128, G], f32, tag=tag, name=tag)

            # at = (1-fy)*vt, ab = fy*vb, cl = (1-fx)*vl, cr = fx*vr
            vt = validity(y0, 0, 63)
            ufy = stile("ufy")
            nc.vector.tensor_scalar(ufy[:], fy[:], -1.0, 1.0, Alu.mult, Alu.add)
            at = stile("at")
            nc.vector.tensor_tensor(at[:], ufy[:], vt[:], Alu.mult)
            vb = validity(y0, -1, 62)
            ab = stile("ab")
            nc.vector.tensor_tensor(ab[:], fy[:], vb[:], Alu.mult)
            vl = validity(x0, 0, 63)
            ufx = stile("ufx")
            nc.vector.tensor_scalar(ufx[:], fx[:], -1.0, 1.0, Alu.mult, Alu.add)
            cl = stile("cl")
            nc.vector.tensor_tensor(cl[:], ufx[:], vl[:], Alu.mult)
            vr = validity(x0, -1, 62)
            cr = stile("cr")
            nc.vector.tensor_tensor(cr[:], fx[:], vr[:], Alu.mult)

            w00 = wtile("w00")
            nc.vector.tensor_tensor(w00[:], at[:], cl[:], Alu.mult)
            w01 = wtile("w01")
            nc.vector.tensor_tensor(w01[:], at[:], cr[:], Alu.mult)
            w10 = wtile("w10")
            nc.vector.tensor_tensor(w10[:], ab[:], cl[:], Alu.mult)
            w11 = wtile("w11")
            nc.vector.tensor_tensor(w11[:], ab[:], cr[:], Alu.mult)

            # indices: idx_t = y0*66 + x0 + 67 (top row start in padded table)
            idxf = stile("idxf")
            nc.vector.scalar_tensor_tensor(idxf[:], y0[:], 66.0, x0[:], Alu.mult, Alu.add)
            nc.vector.tensor_scalar(idxf[:], idxf[:], 67.0, None, Alu.add)
            idx_i32 = spool.tile([128, G], i32, tag="idx_i32")
            nc.vector.tensor_copy(idx_i32[:], idxf[:])
            idx16 = spool.tile([128, G], i16, tag="idx16")
            nc.vector.tensor_copy(idx16[:], idx_i32[:])

            # wrapped index layout [128, G*8] int16: row p (p<16, replicated x8),
            # col g*8+h holds idx of sample s = g*128 + h*16 + p
            wrap_t = wpool.tile([128, G * 8], i16, tag="wrap_t")
            wrv = wrap_t[:].rearrange("p (g h) -> p g h", h=8)
            for h in range(8):
                nc.sync.dma_start(wrv[0:16, :, h], idx16[h * 16:(h + 1) * 16, :])
            for r in range(1, 8):
                nc.sync.dma_start(wrap_t[r * 16:(r + 1) * 16, :], wrap_t[0:16, :])
            if variant == "idx0":
                nc.vector.memset(wrap_t[:], 0)

            # ---- stage 3: per-chunk gather -> weight -> transpose -> matmul ----
            for c in range(NCH):
                gth = gpool.tile([128, GC, 4 * CIN], f16, tag="gth")
                if variant == "dmaonly":
                    src = dataclasses.replace(
                        xt_d.ap(), ap=[[GC * 4 * CIN, 128], [1, GC * 4 * CIN]])
                    nc.sync.dma_start(
                        gth[:].rearrange("p g c -> p (g c)"), src)
                else:
                    nc.gpsimd.dma_gather(
                        gth[:], gsrc, wrap_t[:, c * GC * 8:(c + 1) * GC * 8],
                        SC, SC, 4 * CIN, elem_step=4 * CIN,
                        single_packet=False, queue_num=c % 4,
                    )

                cols = colpool.tile([128, KK * CHUNK_T * 128], f16, tag="cols")
                for kk in range(KK):
                    pt = ps_tr.tile([128, CHUNK_T * 128], f16, tag="pt")
                    for tin in range(CHUNK_T):
                        gl = tin * KK + kk
                        gg = c * GC + gl
                        if variant == "noweight":
                            z4 = zpool.tile([128, 128], f16, tag="z4")
                            nc.vector.tensor_copy(z4[:], gth[:, gl, 0:CIN])
                        else:
                            z1 = zpool.tile([128, 128], f16, tag="z1")
                            nc.vector.tensor_scalar(
                                z1[:], gth[:, gl, 0:CIN], w00[:, gg:gg + 1],
                                None, Alu.mult)
                            z2 = zpool.tile([128, 128], f16, tag="z2")
                            nc.vector.scalar_tensor_tensor(
                                z2[:], gth[:, gl, CIN:2 * CIN], w01[:, gg:gg + 1], z1[:],
                                Alu.mult, Alu.add)
                            z3 = zpool.tile([128, 128], f16, tag="z3")
                            nc.vector.scalar_tensor_tensor(
                                z3[:], gth[:, gl, 2 * CIN:3 * CIN], w10[:, gg:gg + 1], z2[:],
                                Alu.mult, Alu.add)
                            z4 = zpool.tile([128, 128], f16, tag="z4")
                            nc.vector.scalar_tensor_tensor(
                                z4[:], gth[:, gl, 3 * CIN:4 * CIN], w11[:, gg:gg + 1], z3[:],
                                Alu.mult, Alu.add)
                        if variant != "notrans" or tin == 0:
                            nc.tensor.transpose(
                                pt[:, tin * 128:(tin + 1) * 128], z4[:], id16[:])
                    nc.scalar.copy(cols[:, kk * (CHUNK_T * 128):
                                        (kk + 1) * (CHUNK_T * 128)], pt[:])

                for ot in range(2):
                    pm = ps_mm.tile([128, CHUNK_T * 128], f32, tag=f"pm{ot}")
                    for kk in range(KK if variant != "nomm" else 1):
                        nc.tensor.matmul(
                            pm[:],
                            wdct[:, kk * COUT + ot * 128:kk * COUT + (ot + 1) * 128],
                            cols[:, kk * CHUNK_T * 128:(kk + 1) * CHUNK_T * 128],
                            start=(kk == 0),
                            stop=(kk == KK - 1 or variant == "nomm"),
                        )
                    osb = opool.tile([128, CHUNK_T * 128], f16, tag="osb")
                    nc.scalar.activation(
                        osb[:], pm[:], Act.Identity, bias=bdc[:, ot:ot + 1], scale=1.0)
                    nc.sync.dma_start(
                        out_d.ap()[ot * 128:(ot + 1) * 128,
                                   c * CHUNK_T * 128:(c + 1) * CHUNK_T * 128],
                        osb[:])

    nc.compile()
    return nc


def _build_gather_probe(nc, loop_n, variant):
    """Timing probe: just the 16 chunk gathers, nothing else."""
    import concourse.tile as tile
    from concourse import mybir

    f16 = mybir.dt.float16
    f32 = mybir.dt.float32
    i16 = mybir.dt.int16

    xt_d = nc.dram_tensor("xt", [TROWS * 4 * CIN], f16, kind="ExternalInput")
    wrapd_d = nc.dram_tensor("wrapd", [128, G * 8], i16, kind="ExternalInput")
    out_d = nc.dram_tensor("out", [COUT, HW], f32, kind="ExternalOutput")
    gsrc = dataclasses.replace(xt_d.ap(), ap=[[4 * CIN, TROWS - 1], [1, 4 * CIN]])

    from contextlib import ExitStack, nullcontext

    with tile.TileContext(nc) as tc, ExitStack() as ctx:
        cpool = ctx.enter_context(tc.tile_pool(name="consts", bufs=1))
        gpool = ctx.enter_context(tc.tile_pool(name="gath", bufs=4))
        loop_cm = tc.For_i(0, loop_n, 1) if loop_n else nullcontext()
        with loop_cm:
            wrap_t = cpool.tile([128, G * 8], i16)
            nc.sync.dma_start(wrap_t[:], wrapd_d.ap())
            for c in range(NCH):
                gth = gpool.tile([128, GC, 4 * CIN], f16, tag="gth")
                if variant in ("donly", "dhalf"):
                    n = GC * 4 * CIN if variant == "donly" else GC * 2 * CIN
                    src = dataclasses.replace(xt_d.ap(), ap=[[n, 128], [1, n]])
                    nc.sync.dma_start(
                        gth[:].rearrange("p g c -> p (g c)")[:, 0:n], src)
                else:
                    nc.gpsimd.dma_gather(
                        gth[:], gsrc, wrap_t[:, c * GC * 8:(c + 1) * GC * 8],
                        SC, SC, 4 * CIN, elem_step=4 * CIN,
                        single_packet=False,
                        queue_num=(c % 4) if variant == "gonly4q" else 0,
                    )
    nc.compile()
    return nc


def _host_indices(x, w_off, b_off):
    """Host-side replica of the device index math (for gather probes)."""
    KI = np.arange(KK) // 3
    KJ = np.arange(KK) % 3
    idx_all = np.zeros((128, G), np.int16)
    for b in range(1):
        xpad = np.pad(x[b], ((0, 0), (1, 1), (1, 1)), mode="reflect")
        off = np.zeros((2 * KK, H, W), np.float32)
        for kk in range(KK):
            patch = xpad[:, KI[kk]:KI[kk] + H, KJ[kk]:KJ[kk] + W]
            off += np.einsum("oc,chw->ohw",
                             w_off[:, :, KI[kk], KJ[kk]].astype(np.float32),
                             patch.astype(np.float32))
        off += b_off[:, None, None]
        i_g = np.arange(H)[:, None] + np.zeros((1, W))
        j_g = np.zeros((H, 1)) + np.arange(W)[None, :]
        for kk in range(KK):
            py = off[2 * kk] + i_g + KI[kk] - 1
            px = off[2 * kk + 1] + j_g + KJ[kk] - 1
            py = np.clip(py, -1.0, 64.75)
            px = np.clip(px, -1.0, 64.75)
            y0 = np.floor(py).astype(np.int32)
            x0 = np.floor(px).astype(np.int32)
            idx = (y0 * 66 + x0 + 67).astype(np.int16)   # [64, 64]
            idxf = idx.reshape(NT, 128)                   # [t, lp]
            idx_all[:, kk::KK] = idxf.T
    wrapd = np.zeros((128, G * 8), np.int16)
    for h in range(8):
        wrapd[0:16, h::8] = idx_all[h * 16:(h + 1) * 16, :]
    for r in range(1, 8):
        wrapd[r * 16:(r + 1) * 16, :] = wrapd[0:16, :]
    return wrapd


def _host_prep(x, w_off, b_off, w_dc, b_dc):
    """Build per-core input maps (all static layout prep in numpy)."""
    f16 = np.float16
    KI = np.arange(KK) // 3
    KJ = np.arange(KK) % 3

    wofft = np.zeros((CIN, KK * 2 * KK), f16)
    for kk in range(KK):
        wofft[:, kk * 18:(kk + 1) * 18] = w_off[:, :, KI[kk], KJ[kk]].T.astype(f16)
    wdct = np.zeros((CIN, KK * COUT), f16)
    for kk in range(KK):
        wdct[:, kk * COUT:(kk + 1) * COUT] = w_dc[:, :, KI[kk], KJ[kk]].T.astype(f16)
    bdc = np.ascontiguousarray(b_dc.reshape(2, 128).T).astype(np.float32)

    # index bases with b_off folded in: col g = t*9+kk, row lp
    lp = np.arange(128)
    t = np.arange(NT)
    i_of = (t[:, None] * 128 + lp[None, :]) // 64      # [t, lp]
    j_of = (t[:, None] * 128 + lp[None, :]) % 64
    pyb = np.zeros((128, G), np.float32)
    pxb = np.zeros((128, G), np.float32)
    for kk in range(KK):
        pyb[:, kk::KK] = (i_of + KI[kk] - 1 + b_off[2 * kk]).T
        pxb[:, kk::KK] = (j_of + KJ[kk] - 1 + b_off[2 * kk + 1]).T
    id16 = np.eye(128, dtype=f16)
    id32 = np.eye(32, dtype=np.float32)

    common = {
        "wofft": wofft, "wdct": wdct, "bdc": bdc,
        "pyb": pyb, "pxb": pxb, "id16": id16, "id32": id32,
    }
    in_maps = []
    for b in range(B):
        xb = x[b]
        xpad = np.pad(xb, ((0, 0), (1, 1), (1, 1)), mode="reflect").astype(f16)
        # 2x2-patch table: row r=(y2*66+x2) = [xz[y2,x2], xz[y2,x2+1],
        # xz[y2+1,x2], xz[y2+1,x2+1]] each 128 ch; xz = zero-padded image
        xz = np.zeros((68, 68, CIN), f16)
        xz[1:65, 1:65, :] = xb.transpose(1, 2, 0).astype(f16)
        t00 = xz[0:66, 0:66]
        t01 = xz[0:66, 1:67]
        t10 = xz[1:67, 0:66]
        t11 = xz[1:67, 1:67]
        tab = np.concatenate([t00, t01, t10, t11], axis=2).reshape(-1, 4 * CIN)
        tab = np.concatenate(
            [tab, np.zeros((TROWS - 66 * 66, 4 * CIN), f16)], 0)
        in_maps.append({
            "xpad": np.ascontiguousarray(xpad.reshape(CIN, PW * PW)),
            "xt": np.ascontiguousarray(tab.reshape(-1)),
            **common,
        })
    return in_maps


def kernel(x, w_off, b_off, w_dc, b_dc):
    x = np.asarray(x, dtype=np.float32)
    w_off = np.asarray(w_off, dtype=np.float32)
    b_off = np.asarray(b_off, dtype=np.float32)
    w_dc = np.asarray(w_dc, dtype=np.float32)
    b_dc = np.asarray(b_dc, dtype=np.float32)

    if "nc" not in _CACHE:
        _CACHE["nc"] = _build_program()
    nc = _CACHE["nc"]

    from concourse.bass_utils import run_bass_kernel_spmd

    in_maps = _host_prep(x, w_off, b_off, w_dc, b_dc)
    res = run_bass_kernel_spmd(nc, in_maps, list(range(B)))
    out = np.stack([res.results[b]["out"].reshape(COUT, H, W) for b in range(B)])
    return out.astype(np.float32)


if __name__ == "__main__":
    # smoke test with random data
    rng = np.random.default_rng(0)
    x = rng.standard_normal((B, CIN, H, W), dtype=np.float32)
    w_off = rng.standard_normal((2 * KK, CIN, K, K), dtype=np.float32) / 34
    b_off = rng.standard_normal((2 * KK,), dtype=np.float32) * 0.01
    w_dc = rng.standard_normal((COUT, CIN, K, K), dtype=np.float32) / 34
    b_dc = rng.standard_normal((COUT,), dtype=np.float32) * 0.01
    out = kernel(x, w_off, b_off, w_dc, b_dc)
    print("out", out.shape, out.dtype, float(np.abs(out).mean()))
